# revision 9
# baseline (speedup 1.0000x reference)
"""Trainium2 Bass kernel: segment-mean over contextual encodings.

Reference computation:
    emb  = concat([x[:, 257:769, :], broadcast(x[:, 0:1, :])], -1)   # [B, S, 2D]
    out  = scatter_mean(emb by segment_ids[:, 257:769]) -> [2048, 2D]

Sharding strategy (chosen over the batch-parallel hint): shard the OUTPUT
segments across the 8 cores (256 segments each) so no all-reduce is needed.
Host-side sharding partitions the token indices by segment range (metadata
only — all x data movement happens on-device via indirect-DMA gather).

Key algebraic split: output columns [0:1024] need the real segment-sum of
x-window rows (the memory-bound part); columns [1024:2048] are the broadcast
CLS row, whose segment-sum factorizes as per-(segment,batch) counts @ x[:,0,:]
— a tiny [2048,32]@[32,1024] matmul. This halves the data that must move.

Per core: indirect-gather its ~2048 token rows (4KB each) from HBM, build
128-wide one-hot matrices on DVE, accumulate segment sums + per-batch counts
with fp32r matmuls in PSUM, then divide by counts and write its 256-row
output slice.
"""

import numpy as np

B = 32          # batch
TSEQ = 1024     # sequence length of x
D = 1024        # feature dim
SENT = 512
CTX = 256
NSEG = 2048
LO = 1 + CTX    # 257
HI = LO + SENT  # 769
NCORES = 8
SEGS_PER_CORE = NSEG // NCORES   # 256
P = 128
BUCKETS = SEGS_PER_CORE // P     # 2

# fp32 values ship as bf16 (hi, lo) planes: lo = fp32(x) - hi. One-hot
# matmuls then run at full bf16 PE rate (cheap LDWEIGHTS + FWL) while hi+lo
# reconstructs ~17 mantissa bits; PSUM accumulates fp32. Same HBM bytes as fp32.

LAST_RESULTS = None  # BassKernelResults of the most recent run (for test.py)


def _build_shards(seg_flat):
    """Partition token indices by owning core / 128-seg bucket (host-side
    sharding metadata; cheap argwhere/bincount work on 16K ints)."""
    tok = np.nonzero(seg_flat >= 0)[0]
    tseg = seg_flat[tok]
    core_id = tseg // SEGS_PER_CORE
    bucket_id = (tseg % SEGS_PER_CORE) // P
    local_id = tseg % P

    counts = np.zeros((NCORES, BUCKETS), np.int64)
    for c in range(NCORES):
        sel = core_id == c
        for b in range(BUCKETS):
            counts[c, b] = int(np.sum(sel & (bucket_id == b)))
    chunks_per_bucket = max(1, int(-(-counts.max() // P)))
    k_pad = BUCKETS * chunks_per_bucket * P

    idx_arr = np.zeros((NCORES, k_pad), np.int32)       # pad -> row 0 (harmless)
    segl_arr = np.full((NCORES, k_pad), -1.0, np.float32)  # pad -> -1 (one-hot miss)
    bat_arr = np.full((NCORES, k_pad), -1.0, np.float32)
    for c in range(NCORES):
        for b in range(BUCKETS):
            m = (core_id == c) & (bucket_id == b)
            rows = tok[m]
            n = rows.size
            off = b * chunks_per_bucket * P
            idx_arr[c, off:off + n] = rows
            segl_arr[c, off:off + n] = local_id[m]
            bat_arr[c, off:off + n] = rows // SENT
    # transpose to [P, nch] layout (partition-major) for a contiguous DMA
    nch = BUCKETS * chunks_per_bucket
    idx_arr = np.ascontiguousarray(
        idx_arr.reshape(NCORES, nch, P).transpose(0, 2, 1)).reshape(NCORES, -1)
    segl_arr = np.ascontiguousarray(
        segl_arr.reshape(NCORES, nch, P).transpose(0, 2, 1)).reshape(NCORES, -1)
    bat_arr = np.ascontiguousarray(
        bat_arr.reshape(NCORES, nch, P).transpose(0, 2, 1)).reshape(NCORES, -1)
    return chunks_per_bucket, idx_arr, segl_arr, bat_arr


def _build_program(nchunks_per_bucket):
    import concourse.bacc as bacc
    import concourse.bass as bass
    import concourse.tile as tile
    from concourse import mybir
    from concourse.masks import make_identity

    C = nchunks_per_bucket
    f32 = mybir.dt.float32
    bf16 = mybir.dt.bfloat16
    i32 = mybir.dt.int32
    K_pad = BUCKETS * C * P
    D2 = 2 * D  # [hi | lo] bf16 planes per row

    nc = bacc.Bacc("TRN2", target_bir_lowering=False, debug=False,
                   num_devices=NCORES, num_swdge_queues=2)
    xw_d = nc.dram_tensor("xw", [B * SENT, D2], bf16, kind="ExternalInput")
    x0_d = nc.dram_tensor("x0", [B, D2], bf16, kind="ExternalInput")
    idx_d = nc.dram_tensor("idx", [K_pad], i32, kind="ExternalInput")
    segl_d = nc.dram_tensor("segl", [K_pad], f32, kind="ExternalInput")
    bat_d = nc.dram_tensor("bat", [K_pad], f32, kind="ExternalInput")
    out_d = nc.dram_tensor("out", [SEGS_PER_CORE, 2 * D], f32,
                           kind="ExternalOutput")

    with tile.TileContext(nc) as tc:
        with (
            tc.tile_pool(name="const", bufs=1) as constp,
            tc.tile_pool(name="data", bufs=18) as datap,
            tc.tile_pool(name="oh", bufs=18) as ohp,
            tc.tile_pool(name="ep", bufs=2) as epp,
            tc.tile_pool(name="outs", bufs=4) as outsp,
            tc.tile_pool(name="psum", bufs=2, space="PSUM") as psump,
        ):
            iota_i = constp.tile([P, P], i32)
            nc.gpsimd.iota(iota_i[:], pattern=[[1, P]], base=0,
                           channel_multiplier=0)
            iota_f = constp.tile([P, P], f32)
            nc.vector.tensor_copy(out=iota_f[:], in_=iota_i[:])
            iota_b_i = constp.tile([P, B], i32)
            nc.gpsimd.iota(iota_b_i[:], pattern=[[1, B]], base=0,
                           channel_multiplier=0)
            iota_b_f = constp.tile([P, B], f32)
            nc.vector.tensor_copy(out=iota_b_f[:], in_=iota_b_i[:])
            ident = constp.tile([P, P], f32)
            make_identity(nc, ident[:])
            x0_sb = constp.tile([B, D2], bf16)
            nc.sync.dma_start(out=x0_sb[:], in_=x0_d.ap()[:])

            # all chunk metadata in three strided DMAs: SBUF [P, nchunks]
            # where column c holds tokens [c*128, (c+1)*128)
            nch = BUCKETS * C
            idx_all = constp.tile([P, nch], i32)
            nc.sync.dma_start(out=idx_all[:],
                              in_=idx_d.ap().rearrange("(p c) -> p c", p=P))
            segl_all = constp.tile([P, nch], f32)
            nc.sync.dma_start(out=segl_all[:],
                              in_=segl_d.ap().rearrange("(p c) -> p c", p=P))
            bat_all = constp.tile([P, nch], f32)
            nc.sync.dma_start(out=bat_all[:],
                              in_=bat_d.ap().rearrange("(p c) -> p c", p=P))

            # persistent PSUM accumulators: per bucket, segment sums of the
            # x-window half [P, D] and per-batch counts (+ total) [P, B+1]
            acc = [psump.tile([P, D], f32, tag="acc", name=f"acc{i}")
                   for i in range(BUCKETS)]
            cms = [psump.tile([P, B], f32, tag="cm", name=f"cm{i}")
                   for i in range(BUCKETS)]

            for ci in range(BUCKETS * C):
                b = ci // C
                first = (ci % C) == 0
                last = (ci % C) == C - 1

                idx_t = idx_all[:, ci:ci + 1]
                segl_t = segl_all[:, ci:ci + 1]
                bat_t = bat_all[:, ci:ci + 1]

                data_t = datap.tile([P, D2], bf16)
                nc.gpsimd.indirect_dma_start(
                    out=data_t[:],
                    out_offset=None,
                    in_=xw_d.ap()[:],
                    in_offset=bass.IndirectOffsetOnAxis(ap=idx_t,
                                                        axis=0),
                )

                oh_seg = ohp.tile([P, P], bf16, tag="ohseg")
                nc.vector.tensor_tensor(
                    out=oh_seg[:], in0=iota_f[:],
                    in1=segl_t.to_broadcast([P, P]),
                    op=mybir.AluOpType.is_equal)
                oh_b = ohp.tile([P, B], bf16, tag="ohb")
                nc.vector.tensor_tensor(
                    out=oh_b[:], in0=iota_b_f[:],
                    in1=bat_t.to_broadcast([P, B]),
                    op=mybir.AluOpType.is_equal)

                for j in range(2):
                    nc.tensor.matmul(
                        out=acc[b][:, j * 512:(j + 1) * 512],
                        lhsT=oh_seg[:], rhs=data_t[:, j * 512:(j + 1) * 512],
                        start=first, stop=False)
                    nc.tensor.matmul(
                        out=acc[b][:, j * 512:(j + 1) * 512],
                        lhsT=oh_seg[:],
                        rhs=data_t[:, D + j * 512:D + (j + 1) * 512],
                        start=False, stop=last)
                nc.tensor.matmul(
                    out=cms[b][:, :], lhsT=oh_seg[:], rhs=oh_b[:],
                    start=first, stop=last)

            for b in range(BUCKETS):
                cm_sb = epp.tile([P, B], f32, tag="cmsb")
                nc.vector.tensor_copy(out=cm_sb[:], in_=cms[b][:])
                cnt_t = epp.tile([P, 1], f32, tag="cnt")
                nc.vector.tensor_reduce(out=cnt_t[:], in_=cm_sb[:],
                                        axis=mybir.AxisListType.X,
                                        op=mybir.AluOpType.add)
                nc.vector.tensor_scalar_max(out=cnt_t[:], in0=cnt_t[:],
                                            scalar1=1.0)
                recip_t = epp.tile([P, 1], f32, tag="recip")
                nc.vector.reciprocal(out=recip_t[:], in_=cnt_t[:])

                # x-window half: divide by counts, write out. Do this before
                # the cls matmul so acc[b]'s PSUM slot frees for cls_ps.
                o1 = outsp.tile([P, D], f32, tag="o")
                nc.vector.tensor_scalar_mul(out=o1[:], in0=acc[b][:],
                                            scalar1=recip_t[:, 0:1])
                nc.sync.dma_start(out=out_d.ap()[b * P:(b + 1) * P, 0:D],
                                  in_=o1[:])

                # cls half: counts[P segs, B].T via PE transpose, then
                # [B, P].T @ x0 [B, D] accumulated in PSUM.
                trp = psump.tile([B, P], f32, tag="tr")
                nc.tensor.transpose(out=trp[:], in_=cm_sb[:, 0:B],
                                    identity=ident[:])
                cmT_sb = epp.tile([B, P], bf16, tag="cmT")
                nc.vector.tensor_copy(out=cmT_sb[:], in_=trp[:])

                cls_ps = psump.tile([P, D], f32, tag="acc")
                for j in range(2):
                    nc.tensor.matmul(
                        out=cls_ps[:, j * 512:(j + 1) * 512],
                        lhsT=cmT_sb[:],
                        rhs=x0_sb[:, j * 512:(j + 1) * 512],
                        start=True, stop=False)
                    nc.tensor.matmul(
                        out=cls_ps[:, j * 512:(j + 1) * 512],
                        lhsT=cmT_sb[:],
                        rhs=x0_sb[:, D + j * 512:D + (j + 1) * 512],
                        start=False, stop=True)

                o2 = outsp.tile([P, D], f32, tag="o")
                nc.scalar.activation(out=o2[:], in_=cls_ps[:],
                                     func=mybir.ActivationFunctionType.Copy,
                                     scale=recip_t[:, 0:1])
                nc.sync.dma_start(out=out_d.ap()[b * P:(b + 1) * P, D:2 * D],
                                  in_=o2[:])

    nc.compile()
    return nc


def _split_hilo(a32):
    """fp32 [N, D] -> bf16 [N, 2D]: hi plane | lo plane, lo = x - fp32(hi)."""
    import ml_dtypes
    hi = a32.astype(ml_dtypes.bfloat16)
    lo = (a32 - hi.astype(np.float32)).astype(ml_dtypes.bfloat16)
    out = np.empty((a32.shape[0], 2 * a32.shape[1]), dtype=ml_dtypes.bfloat16)
    out[:, :a32.shape[1]] = hi
    out[:, a32.shape[1]:] = lo
    return out


def kernel(x, segment_ids):
    global LAST_RESULTS
    from concourse.bass_utils import run_bass_kernel_spmd

    x = np.asarray(x, dtype=np.float32)
    seg_all = np.asarray(segment_ids).astype(np.int64)
    assert x.shape == (B, TSEQ, D), x.shape
    assert seg_all.shape == (B, TSEQ), seg_all.shape

    xw = _split_hilo(np.ascontiguousarray(x[:, LO:HI, :].reshape(B * SENT, D)))
    x0 = _split_hilo(np.ascontiguousarray(x[:, 0, :]))
    seg_flat = seg_all[:, LO:HI].reshape(-1)

    C, idx_arr, segl_arr, bat_arr = _build_shards(seg_flat)
    nc = _build_program(C)

    in_maps = [
        {"xw": xw, "x0": x0, "idx": idx_arr[c], "segl": segl_arr[c],
         "bat": bat_arr[c]}
        for c in range(NCORES)
    ]
    last_err = None
    for _attempt in range(3):
        try:
            res = run_bass_kernel_spmd(nc, in_maps, list(range(NCORES)))
            break
        except Exception as e:  # transient NRT device errors happen; retry
            last_err = e
    else:
        raise last_err
    LAST_RESULTS = res
    return np.concatenate([res.results[c]["out"] for c in range(NCORES)],
                          axis=0)


# revision 11
# speedup vs baseline: 1.0902x; 1.0902x over previous
"""Trainium2 Bass kernel: segment-mean over contextual encodings.

Reference computation:
    emb  = concat([x[:, 257:769, :], broadcast(x[:, 0:1, :])], -1)   # [B, S, 2D]
    out  = scatter_mean(emb by segment_ids[:, 257:769]) -> [2048, 2D]

Sharding strategy (chosen over the batch-parallel hint): shard the OUTPUT
segments across the 8 cores (256 segments each) so no all-reduce is needed.
Host-side sharding partitions the token indices by segment range (metadata
only — all x data movement happens on-device via indirect-DMA gather).

Key algebraic split: output columns [0:1024] need the real segment-sum of
x-window rows (the memory-bound part); columns [1024:2048] are the broadcast
CLS row, whose segment-sum factorizes as per-(segment,batch) counts @ x[:,0,:]
— a tiny [2048,32]@[32,1024] matmul. This halves the data that must move.

Per core: indirect-gather its ~2048 token rows (4KB each) from HBM, build
128-wide one-hot matrices on DVE, accumulate segment sums + per-batch counts
with fp32r matmuls in PSUM, then divide by counts and write its 256-row
output slice.
"""

import numpy as np

B = 32          # batch
TSEQ = 1024     # sequence length of x
D = 1024        # feature dim
SENT = 512
CTX = 256
NSEG = 2048
LO = 1 + CTX    # 257
HI = LO + SENT  # 769
NCORES = 8
SEGS_PER_CORE = NSEG // NCORES   # 256
P = 128
BUCKETS = SEGS_PER_CORE // P     # 2

# fp32 values ship as bf16 (hi, lo) planes: lo = fp32(x) - hi. One-hot
# matmuls then run at full bf16 PE rate (cheap LDWEIGHTS + FWL) while hi+lo
# reconstructs ~17 mantissa bits; PSUM accumulates fp32. Same HBM bytes as fp32.

LAST_RESULTS = None  # BassKernelResults of the most recent run (for test.py)


def _build_shards(seg_flat):
    """Partition token indices by owning core / 128-seg bucket (host-side
    sharding metadata; cheap argwhere/bincount work on 16K ints)."""
    tok = np.nonzero(seg_flat >= 0)[0]
    tseg = seg_flat[tok]
    core_id = tseg // SEGS_PER_CORE
    bucket_id = (tseg % SEGS_PER_CORE) // P
    local_id = tseg % P

    counts = np.zeros((NCORES, BUCKETS), np.int64)
    for c in range(NCORES):
        sel = core_id == c
        for b in range(BUCKETS):
            counts[c, b] = int(np.sum(sel & (bucket_id == b)))
    chunks_per_bucket = max(1, int(-(-counts.max() // P)))
    k_pad = BUCKETS * chunks_per_bucket * P

    idx_arr = np.zeros((NCORES, k_pad), np.int32)       # pad -> row 0 (harmless)
    segl_arr = np.full((NCORES, k_pad), -1.0, np.float32)  # pad -> -1 (one-hot miss)
    bat_arr = np.full((NCORES, k_pad), -1.0, np.float32)
    for c in range(NCORES):
        for b in range(BUCKETS):
            m = (core_id == c) & (bucket_id == b)
            rows = tok[m]
            n = rows.size
            off = b * chunks_per_bucket * P
            idx_arr[c, off:off + n] = rows
            segl_arr[c, off:off + n] = local_id[m]
            bat_arr[c, off:off + n] = rows // SENT
    # transpose to [P, nch] layout (partition-major) for a contiguous DMA
    nch = BUCKETS * chunks_per_bucket
    idx_arr = np.ascontiguousarray(
        idx_arr.reshape(NCORES, nch, P).transpose(0, 2, 1)).reshape(NCORES, -1)
    segl_arr = np.ascontiguousarray(
        segl_arr.reshape(NCORES, nch, P).transpose(0, 2, 1)).reshape(NCORES, -1)
    bat_arr = np.ascontiguousarray(
        bat_arr.reshape(NCORES, nch, P).transpose(0, 2, 1)).reshape(NCORES, -1)
    return chunks_per_bucket, idx_arr, segl_arr, bat_arr


def _build_program(nchunks_per_bucket):
    import concourse.bacc as bacc
    import concourse.bass as bass
    import concourse.tile as tile
    from concourse import mybir
    from concourse.masks import make_identity

    C = nchunks_per_bucket
    f32 = mybir.dt.float32
    bf16 = mybir.dt.bfloat16
    i32 = mybir.dt.int32
    K_pad = BUCKETS * C * P
    D2 = 2 * D  # [hi | lo] bf16 planes per row

    nc = bacc.Bacc("TRN2", target_bir_lowering=False, debug=False,
                   num_devices=NCORES)
    xw_d = nc.dram_tensor("xw", [B * SENT, D2], bf16, kind="ExternalInput")
    x0_d = nc.dram_tensor("x0", [B, D2], bf16, kind="ExternalInput")
    idx_d = nc.dram_tensor("idx", [K_pad], i32, kind="ExternalInput")
    segl_d = nc.dram_tensor("segl", [K_pad], f32, kind="ExternalInput")
    bat_d = nc.dram_tensor("bat", [K_pad], f32, kind="ExternalInput")
    out_d = nc.dram_tensor("out", [SEGS_PER_CORE, 2 * D], f32,
                           kind="ExternalOutput")

    with tile.TileContext(nc) as tc:
        with (
            tc.tile_pool(name="const", bufs=1) as constp,
            tc.tile_pool(name="data", bufs=18) as datap,
            tc.tile_pool(name="oh", bufs=18) as ohp,
            tc.tile_pool(name="ep", bufs=2) as epp,
            tc.tile_pool(name="outs", bufs=4) as outsp,
            tc.tile_pool(name="psum", bufs=2, space="PSUM") as psump,
        ):
            # gather index metadata first: the serialized Q7 gather stream
            # gates everything, so it must start as early as possible
            nch = BUCKETS * C
            idx_all = constp.tile([P, nch], i32)
            nc.sync.dma_start(out=idx_all[:],
                              in_=idx_d.ap().rearrange("(p c) -> p c", p=P))

            # persistent PSUM accumulators: per bucket, segment sums of the
            # x-window half [P, D] and per-batch counts (+ total) [P, B+1]
            acc = [psump.tile([P, D], f32, tag="acc", name=f"acc{i}")
                   for i in range(BUCKETS)]
            cms = [psump.tile([P, B], f32, tag="cm", name=f"cm{i}")
                   for i in range(BUCKETS)]

            # all gathers issued up-front (program order sets Tile priority;
            # the serialized Q7 descriptor-gen stream gates the kernel)
            gathered = []
            for g0 in range(BUCKETS * C):
                dt_g = datap.tile([P, D2], bf16, tag="data", name=f"g{g0}")
                nc.gpsimd.indirect_dma_start(
                    out=dt_g[:],
                    out_offset=None,
                    in_=xw_d.ap()[:],
                    in_offset=bass.IndirectOffsetOnAxis(
                        ap=idx_all[:, g0:g0 + 1], axis=0),
                )
                gathered.append(dt_g)

            # constants + remaining metadata (overlap with the gather stream)
            iota_i = constp.tile([P, P], i32)
            nc.gpsimd.iota(iota_i[:], pattern=[[1, P]], base=0,
                           channel_multiplier=0)
            iota_f = constp.tile([P, P], f32)
            nc.vector.tensor_copy(out=iota_f[:], in_=iota_i[:])
            iota_b_i = constp.tile([P, B], i32)
            nc.gpsimd.iota(iota_b_i[:], pattern=[[1, B]], base=0,
                           channel_multiplier=0)
            iota_b_f = constp.tile([P, B], f32)
            nc.vector.tensor_copy(out=iota_b_f[:], in_=iota_b_i[:])
            ident = constp.tile([P, P], f32)
            make_identity(nc, ident[:])
            x0_sb = constp.tile([B, D2], bf16)
            nc.sync.dma_start(out=x0_sb[:], in_=x0_d.ap()[:])
            segl_all = constp.tile([P, nch], f32)
            nc.sync.dma_start(out=segl_all[:],
                              in_=segl_d.ap().rearrange("(p c) -> p c", p=P))
            bat_all = constp.tile([P, nch], f32)
            nc.sync.dma_start(out=bat_all[:],
                              in_=bat_d.ap().rearrange("(p c) -> p c", p=P))

            for ci in range(BUCKETS * C):
                b = ci // C
                first = (ci % C) == 0
                last = (ci % C) == C - 1

                segl_t = segl_all[:, ci:ci + 1]
                bat_t = bat_all[:, ci:ci + 1]
                data_t = gathered[ci]

                oh_seg = ohp.tile([P, P], bf16, tag="ohseg")
                nc.vector.tensor_tensor(
                    out=oh_seg[:], in0=iota_f[:],
                    in1=segl_t.to_broadcast([P, P]),
                    op=mybir.AluOpType.is_equal)
                oh_b = ohp.tile([P, B], bf16, tag="ohb")
                nc.vector.tensor_tensor(
                    out=oh_b[:], in0=iota_b_f[:],
                    in1=bat_t.to_broadcast([P, B]),
                    op=mybir.AluOpType.is_equal)

                for j in range(2):
                    nc.tensor.matmul(
                        out=acc[b][:, j * 512:(j + 1) * 512],
                        lhsT=oh_seg[:], rhs=data_t[:, j * 512:(j + 1) * 512],
                        start=first, stop=False)
                    nc.tensor.matmul(
                        out=acc[b][:, j * 512:(j + 1) * 512],
                        lhsT=oh_seg[:],
                        rhs=data_t[:, D + j * 512:D + (j + 1) * 512],
                        start=False, stop=last)
                nc.tensor.matmul(
                    out=cms[b][:, :], lhsT=oh_seg[:], rhs=oh_b[:],
                    start=first, stop=last)

            for b in range(BUCKETS):
                cm_sb = epp.tile([P, B], f32, tag="cmsb")
                nc.vector.tensor_copy(out=cm_sb[:], in_=cms[b][:])
                cnt_t = epp.tile([P, 1], f32, tag="cnt")
                nc.vector.tensor_reduce(out=cnt_t[:], in_=cm_sb[:],
                                        axis=mybir.AxisListType.X,
                                        op=mybir.AluOpType.add)
                nc.vector.tensor_scalar_max(out=cnt_t[:], in0=cnt_t[:],
                                            scalar1=1.0)
                recip_t = epp.tile([P, 1], f32, tag="recip")
                nc.vector.reciprocal(out=recip_t[:], in_=cnt_t[:])

                # x-window half: divide by counts, write out. Do this before
                # the cls matmul so acc[b]'s PSUM slot frees for cls_ps.
                o1 = outsp.tile([P, D], f32, tag="o")
                nc.vector.tensor_scalar_mul(out=o1[:], in0=acc[b][:],
                                            scalar1=recip_t[:, 0:1])
                nc.sync.dma_start(out=out_d.ap()[b * P:(b + 1) * P, 0:D],
                                  in_=o1[:])

                # cls half: counts[P segs, B].T via PE transpose, then
                # [B, P].T @ x0 [B, D] accumulated in PSUM.
                trp = psump.tile([B, P], f32, tag="tr")
                nc.tensor.transpose(out=trp[:], in_=cm_sb[:, 0:B],
                                    identity=ident[:])
                cmT_sb = epp.tile([B, P], bf16, tag="cmT")
                nc.vector.tensor_copy(out=cmT_sb[:], in_=trp[:])

                cls_ps = psump.tile([P, D], f32, tag="acc")
                for j in range(2):
                    nc.tensor.matmul(
                        out=cls_ps[:, j * 512:(j + 1) * 512],
                        lhsT=cmT_sb[:],
                        rhs=x0_sb[:, j * 512:(j + 1) * 512],
                        start=True, stop=False)
                    nc.tensor.matmul(
                        out=cls_ps[:, j * 512:(j + 1) * 512],
                        lhsT=cmT_sb[:],
                        rhs=x0_sb[:, D + j * 512:D + (j + 1) * 512],
                        start=False, stop=True)

                o2 = outsp.tile([P, D], f32, tag="o")
                nc.scalar.activation(out=o2[:], in_=cls_ps[:],
                                     func=mybir.ActivationFunctionType.Copy,
                                     scale=recip_t[:, 0:1])
                nc.sync.dma_start(out=out_d.ap()[b * P:(b + 1) * P, D:2 * D],
                                  in_=o2[:])

    nc.compile()
    return nc


def _split_hilo(a32):
    """fp32 [N, D] -> bf16 [N, 2D]: hi plane | lo plane, lo = x - fp32(hi)."""
    import ml_dtypes
    hi = a32.astype(ml_dtypes.bfloat16)
    lo = (a32 - hi.astype(np.float32)).astype(ml_dtypes.bfloat16)
    out = np.empty((a32.shape[0], 2 * a32.shape[1]), dtype=ml_dtypes.bfloat16)
    out[:, :a32.shape[1]] = hi
    out[:, a32.shape[1]:] = lo
    return out


def kernel(x, segment_ids):
    global LAST_RESULTS
    from concourse.bass_utils import run_bass_kernel_spmd

    x = np.asarray(x, dtype=np.float32)
    seg_all = np.asarray(segment_ids).astype(np.int64)
    assert x.shape == (B, TSEQ, D), x.shape
    assert seg_all.shape == (B, TSEQ), seg_all.shape

    xw = _split_hilo(np.ascontiguousarray(x[:, LO:HI, :].reshape(B * SENT, D)))
    x0 = _split_hilo(np.ascontiguousarray(x[:, 0, :]))
    seg_flat = seg_all[:, LO:HI].reshape(-1)

    C, idx_arr, segl_arr, bat_arr = _build_shards(seg_flat)
    nc = _build_program(C)

    in_maps = [
        {"xw": xw, "x0": x0, "idx": idx_arr[c], "segl": segl_arr[c],
         "bat": bat_arr[c]}
        for c in range(NCORES)
    ]
    last_err = None
    for _attempt in range(3):
        try:
            res = run_bass_kernel_spmd(nc, in_maps, list(range(NCORES)))
            break
        except Exception as e:  # transient NRT device errors happen; retry
            last_err = e
    else:
        raise last_err
    LAST_RESULTS = res
    return np.concatenate([res.results[c]["out"] for c in range(NCORES)],
                          axis=0)
